# revision 1
# baseline (speedup 1.0000x reference)
"""Distributed GNN (4-layer GraphConv) Bass kernel for 8 TRN2 NeuronCores.

Self-contained: hosts the graph preprocessing (balanced node->window
placement via batched greedy + refinement, per-(chunk,window) uniform-K
gather schedule), the Bass/Tile program (windowed int16 dma_gather +
strided DVE segment reduce + PE transposes/matmuls + ACT bias/relu/
sigmoid, AllGather per layer), and the SPMD orchestration.

The device program keeps all arithmetic in f32 (PE f32 matmuls; tables,
aggregates and weights f32); the only quantization is the 9-bit
per-node-scaled input shard -- max rel err vs the f64 reference ~8e-3.

Host->device traffic is minimized: per core we ship only its node shard
packed to 9 bits/value (0.9MB), a 16-partition gather-index table
(device replicates it to 128 partitions), the deg_inv and scale
vectors, and the small weights -- all in one blob. The device unpacks
and dequantizes x with DVE integer ops, assembles the full f32 gather
table via AllGather, and later layers AllGather their own activations.

kernel(**inputs) takes the FULL unsharded inputs of reference.setup_inputs()
and returns the FULL [100000, 1] float32 output.
"""
import numpy as np
import ml_dtypes

from concourse import bass, bacc, tile, mybir
from concourse.masks import make_identity
from concourse.bass_utils import run_bass_kernel_spmd

N = 100000
E = 1600000
D = 64
NC = 8
NLR = 12500
NLP = 12544          # 98 * 128
V = NC * NLP         # 100352
W = 4
WINP = 2 * NLP       # 25088 rows per gather window (< 32768: int16-safe)
NT = NLP // 128      # 98 tiles
SLOT_BUDGET = 128
T_MAX = 8

F32 = mybir.dt.float32
BF16 = mybir.dt.bfloat16
I16 = mybir.dt.int16
AF = mybir.ActivationFunctionType
ALU = mybir.AluOpType


# ---------------------------------------------------------------- planning
def _batch_edges(batch, starts, d_sorted):
    """Edges of `batch` nodes: (dst array, batch-position per edge, counts)."""
    cnts_all = starts[batch + 1] - starts[batch]
    nzpos = np.where(cnts_all > 0)[0]
    if not len(nzpos):
        return np.empty(0, np.int64), np.empty(0, np.int64), cnts_all
    bsub = batch[nzpos]
    cnts = cnts_all[nzpos]
    st = starts[bsub]
    out = np.ones(int(cnts.sum()), np.int64)
    out[0] = st[0]
    if len(bsub) > 1:
        idx = np.cumsum(cnts)[:-1]
        out[idx] = st[1:] - (st[:-1] + cnts[:-1] - 1)
    eidx = np.cumsum(out)
    seg = nzpos[np.repeat(np.arange(len(bsub)), cnts)]
    return d_sorted[eidx], seg, cnts_all


def _assign_windows(starts, d_sorted, outdeg, deg_in):
    """Balanced node->window placement: batched greedy on sum-of-counts
    score (marginal of sum C^2), then batched f=c^2 refinement, then a
    capacity fixup. Returns win_of[N]."""
    node_order = np.argsort(-outdeg, kind="stable")
    C = np.zeros((N, W), np.int32)
    win_of = np.zeros(N, np.int8)
    cap_used = np.zeros(W, np.int64)
    capmax = 2 * NLR
    B = 4096
    for b0 in range(0, N, B):
        batch = node_order[b0:b0 + B]
        nb = len(batch)
        ds, seg, cnts = _batch_edges(batch, starts, d_sorted)
        scores = np.empty((nb, W), np.float64)
        for w in range(W):
            scores[:, w] = np.bincount(seg, weights=C[ds, w], minlength=nb)
        # round-robin tiebreak + soft capacity pressure
        rows = (np.arange(nb) + b0) % W
        scores[np.arange(nb), rows] -= 0.25
        scores += cap_used[None, :] * (0.5 / capmax)
        full = cap_used >= capmax
        if full.any():
            scores[:, full] = 1e18
        w = np.argmin(scores, axis=1).astype(np.int8)
        win_of[batch] = w
        if len(ds):
            upd = np.bincount(ds * W + w.astype(np.int64)[seg],
                              minlength=N * W)
            C += upd.reshape(N, W).astype(np.int32)
        cap_used += np.bincount(w, minlength=W)

    # refinement: move nodes when it lowers sum of c^2
    for _ in range(2):
        moved = 0
        for b0 in range(0, N, B):
            batch = np.arange(b0, min(b0 + B, N))
            ds, seg, cnts = _batch_edges(batch, starts, d_sorted)
            if not len(ds):
                continue
            nb = len(batch)
            cur = win_of[batch].astype(np.int64)
            gain_out = np.bincount(
                seg, weights=2 * C[ds, cur[seg]] - 1, minlength=nb)
            cost_in = np.empty((nb, W), np.float64)
            for w in range(W):
                cost_in[:, w] = np.bincount(
                    seg, weights=2 * C[ds, w] + 1, minlength=nb)
            full = cap_used >= capmax
            if full.any():
                cost_in[:, full] = 1e18
            cost_in[np.arange(nb), cur] = gain_out
            w1 = np.argmin(cost_in, axis=1)
            improve = cost_in[np.arange(nb), w1] < gain_out - 1e-9
            if not improve.any():
                continue
            mnodes = batch[improve]
            mw0 = win_of[mnodes].astype(np.int64)
            mw1 = w1[improve].astype(np.int8)
            emask = improve[seg]
            ds_m = ds[emask]
            seg_m = seg[emask]
            dec = np.bincount(ds_m * W + cur[seg_m], minlength=N * W)
            inc = np.bincount(ds_m * W + w1[seg_m], minlength=N * W)
            C += (inc.reshape(N, W) - dec.reshape(N, W)).astype(np.int32)
            cap_used += (np.bincount(mw1, minlength=W)
                         - np.bincount(mw0, minlength=W))
            win_of[mnodes] = mw1
            moved += len(mnodes)
        if moved == 0:
            break

    # capacity fixup: windows must hold <= 2*NLR nodes. Total capacity is
    # exactly N, so draining over-full windows into least-full converges;
    # greedy-by-cost while it lasts, arbitrary moves as a safety net.
    for it in range(64 * W):
        over_w = np.where(cap_used > capmax)[0]
        if not len(over_w):
            break
        w = int(over_w[np.argmax(cap_used[over_w])])
        over = int(cap_used[w] - capmax)
        wt = int(np.argmin(cap_used))
        room = int(capmax - cap_used[wt])
        k = max(1, min(over, room))
        nodes_w = np.where(win_of == w)[0]
        if it < 8 * W:
            ds, seg, cnts = _batch_edges(nodes_w, starts, d_sorted)
            dc = np.zeros(len(nodes_w), np.float64)
            if len(ds):
                np.add.at(dc, seg, (2 * C[ds, wt] + 1) - (2 * C[ds, w] - 1))
            sel = np.argpartition(dc, min(k, len(nodes_w) - 1))[:k]
        else:
            sel = np.arange(k)
        movers = nodes_w[sel]
        ds_m, _, _ = _batch_edges(movers, starts, d_sorted)
        if len(ds_m):
            np.add.at(C, (ds_m, w), -1)
            np.add.at(C, (ds_m, wt), 1)
        win_of[movers] = wt
        cap_used[w] -= k
        cap_used[wt] += k
    return win_of, C


def build_plan(edge_index):
    src = np.asarray(edge_index[0], dtype=np.int64)
    dst = np.asarray(edge_index[1], dtype=np.int64)
    deg_in = np.bincount(dst, minlength=N).astype(np.int64)

    order = np.argsort(src, kind="stable")
    s_sorted = src[order]
    d_sorted = dst[order]
    starts = np.searchsorted(s_sorted, np.arange(N + 1))
    outdeg = starts[1:] - starts[:-1]

    win_of, C = _assign_windows(starts, d_sorted, outdeg, deg_in)

    # per-window snake split, sorted by (max window count, degree) desc
    maxc = C.max(axis=1).astype(np.int64)
    sort_key = ((63 - np.minimum(maxc, 63)) * 1024
                + (1023 - np.minimum(deg_in, 1023)))
    gperm = np.empty(N, np.int64)
    orig_of = np.full(V, -1, np.int64)
    for w in range(W):
        nodes_w = np.where(win_of == w)[0]
        order_w = nodes_w[np.argsort(sort_key[nodes_w], kind="stable")]
        for half, core in ((0, 2 * w), (1, 2 * w + 1)):
            sel = order_w[half::2]
            ranks = np.arange(sel.shape[0])
            gperm[sel] = core * NLP + ranks
            orig_of[core * NLP + ranks] = sel

    src_p = gperm[src]
    dst_p = gperm[dst]
    win_s = src_p // WINP
    rel_s = (src_p - win_s * WINP).astype(np.int32)

    cnt = np.bincount(dst_p * W + win_s, minlength=V * W).reshape(NC, NLP, W)
    tile_max = cnt.reshape(NC, NT, 128, W).max(axis=(0, 2))

    chunks = []
    t0 = 0
    while t0 < NT:
        T = 1
        K = tile_max[t0].copy()
        while T < T_MAX and t0 + T < NT:
            K2 = np.maximum(K, tile_max[t0 + T])
            if (T + 1) * int(K2.sum()) > SLOT_BUDGET:
                break
            K = K2
            T += 1
        chunks.append((t0, T, [int(k) for k in K]))
        t0 += T

    ek = dst_p * W + win_s
    eorder = np.argsort(ek, kind="stable")
    ek_s = ek[eorder]
    rel_s_s = rel_s[eorder]
    gstarts = np.searchsorted(ek_s, ek_s)
    kpos = np.arange(E) - gstarts

    core_e = (ek_s // W) // NLP
    rank_e = (ek_s // W) % NLP
    win_e = ek_s % W

    chunk_of_tile = np.empty(NT, np.int32)
    tinc_of_tile = np.empty(NT, np.int32)
    ftot = 0
    call_meta = []
    for ci, (tile0, T, K) in enumerate(chunks):
        chunk_of_tile[tile0:tile0 + T] = ci
        tinc_of_tile[tile0:tile0 + T] = np.arange(T)
        for w in range(W):
            n_idx = 128 * T * K[w]
            call_meta.append(dict(chunk=ci, w=w, tile0=tile0, T=T, K=K[w],
                                  ioff=ftot, n_idx=n_idx))
            ftot += n_idx // 16

    tile_e = rank_e // 128
    p_e = rank_e % 128
    ci_e = chunk_of_tile[tile_e]
    tin_e = tinc_of_tile[tile_e]
    ioff_arr = np.zeros((len(chunks), W), np.int64)
    K_arr = np.zeros((len(chunks), W), np.int64)
    for m in call_meta:
        ioff_arr[m["chunk"], m["w"]] = m["ioff"]
        K_arr[m["chunk"], m["w"]] = m["K"]
    Kk = K_arr[ci_e, win_e]
    j = (tin_e * Kk + kpos) * 128 + p_e
    col = ioff_arr[ci_e, win_e] + j // 16
    row = j % 16

    gidx = np.full((NC, 16, ftot), np.int16(NLR), np.int16)  # pad: zero row
    gidx[core_e, row, col] = rel_s_s.astype(np.int16)

    deg_inv = (1.0 / np.maximum(deg_in, 1)).astype(np.float32)
    deg_inv_perm = np.zeros(V, np.float32)
    deg_inv_perm[gperm] = deg_inv
    deg_inv_perm[orig_of < 0] = 1.0

    return dict(gperm=gperm, orig_of=orig_of, chunks=chunks,
                call_meta=call_meta, ftot=ftot, gidx=gidx,
                deg_inv_perm=deg_inv_perm)


# ---------------------------------------------------------------- program
def build_program(plan, n_cores=NC):
    chunks = plan["chunks"]
    call_meta = plan["call_meta"]
    ftot = plan["ftot"]

    nc = bacc.Bacc("TRN2", target_bir_lowering=False, debug=False,
                   num_devices=n_cores, num_swdge_queues=2)

    # single packed input per core (each device_put/RPC has ~60-80ms fixed
    # cost on the axon tunnel, so everything ships as one f32 blob):
    # [x low bytes u8 | x high nibbles u8 | gidx i16 | dgc f32 | wstk f32 |
    #  bias f32 | x per-node scale f32], f32-word offsets. x is 12-bit
    # per-node-scaled (offset-2048 unsigned); nibble byte j of a node packs
    # features j and j+32 so unpack needs only contiguous slices.
    XL_W = NLP * D // 4
    XH_W = NLP * D // 32
    GI_W = 8 * ftot                  # 16*ftot int16 = 8*ftot f32 words
    DG_W = 128 * NT
    WS_W = D * 8 * D
    BI_W = D * 4
    AP_W = 128 * NT
    xl0 = 0
    xh0 = xl0 + XL_W
    gi0 = xh0 + XH_W
    dg0 = gi0 + GI_W
    ws0 = dg0 + DG_W
    bi0 = ws0 + WS_W
    ap0 = bi0 + BI_W
    totw = ap0 + AP_W
    blob = nc.dram_tensor("blob", [totw, 1], F32, kind="ExternalInput")
    out = nc.dram_tensor("out", [1, NLP], F32, kind="ExternalOutput")

    with tile.TileContext(nc) as tc:
        with tc.tile_pool(name="const", bufs=1) as constp, \
             tc.tile_pool(name="hload", bufs=3) as hloadp, \
             tc.tile_pool(name="unp", bufs=1) as unp, \
             tc.tile_pool(name="msg", bufs=2) as msgp, \
             tc.tile_pool(name="part", bufs=2) as partp, \
             tc.tile_pool(name="agg", bufs=2) as aggp, \
             tc.tile_pool(name="rhs", bufs=3) as rhsp, \
             tc.tile_pool(name="zsb", bufs=3) as zsbp, \
             tc.tile_pool(name="zN", bufs=2) as zNp, \
             tc.tile_pool(name="psA", bufs=2, space="PSUM") as psA, \
             tc.tile_pool(name="psB", bufs=2, space="PSUM") as psB, \
             tc.tile_pool(name="psC", bufs=2, space="PSUM") as psC, \
             tc.tile_pool(name="psD", bufs=2, space="PSUM") as psD, \
             tc.tile_pool(name="dram", bufs=1, space="DRAM") as dramp:

            identf = constp.tile([128, 128], F32)
            make_identity(nc, identf[:])
            wstk_sb = constp.tile([D, 8 * D], F32)
            nc.sync.dma_start(
                out=wstk_sb[:],
                in_=blob.ap()[ws0:ws0 + WS_W, :].rearrange(
                    "(r c) o -> r (c o)", r=D))
            bias_sb = constp.tile([D, 4], F32)
            nc.sync.dma_start(
                out=bias_sb[:],
                in_=blob.ap()[bi0:bi0 + BI_W, :].rearrange(
                    "(r c) o -> r (c o)", r=D))
            dgc_sb = constp.tile([128, NT], F32)
            nc.sync.dma_start(
                out=dgc_sb[:],
                in_=blob.ap()[dg0:dg0 + DG_W, :].rearrange(
                    "(p t) o -> p (t o)", p=128))

            # gather-index table: load 16 rows, replicated to 128
            gidx_ap = blob.ap()[gi0:gi0 + GI_W, :].bitcast(I16).rearrange(
                "(r ch) two -> r (ch two)", r=16)
            idx_sb = constp.tile([128, ftot], I16)
            for k in range(8):
                nc.sync.dma_start(out=idx_sb[16 * k:16 * (k + 1), :],
                                  in_=gidx_ap)

            # deg_inv broadcast [128, NT] -> [128, NT*D]; deg_inv > 0 so
            # Relu(0*x + deginv) == deginv (Copy rejects AP bias)
            dgb = constp.tile([128, NT * D], F32)
            for t in range(NT):
                nc.scalar.activation(
                    out=dgb[:, t * D:(t + 1) * D], in_=identf[:, 0:D],
                    func=AF.Relu, scale=0.0, bias=dgc_sb[:, t:t + 1])

            # x per-node dequant scale (a > 0); broadcast per unpack chunk
            apk_sb = constp.tile([128, NT], F32)
            nc.sync.dma_start(
                out=apk_sb[:],
                in_=blob.ap()[ap0:ap0 + AP_W, :].rearrange(
                    "(p t) o -> p (t o)", p=128))

            agins = [dramp.tile([NLP, D], F32, name=f"agin{i}")
                     for i in range(2)]
            tabs = [dramp.tile([V, D], F32, name=f"tab{i}",
                               addr_space="Shared") for i in range(4)]

            # unpack 12-bit x -> f32 agin1, AllGather -> tab0
            U8 = mybir.dt.uint8
            CH = 14
            QD = D // 8
            g0 = 0
            while g0 < NT:
                Tg = min(CH, NT - g0)
                l8 = unp.tile([128, CH * D], U8, tag="l8")
                nc.sync.dma_start(
                    out=l8[:, :Tg * D].rearrange("p (t f) -> p t f", t=Tg),
                    in_=blob.ap()[xl0 + g0 * 2048:
                                  xl0 + (g0 + Tg) * 2048, :].bitcast(
                        U8).rearrange("(t p fq) four -> p t (fq four)",
                                      p=128, fq=D // 4),
                )
                h8 = unp.tile([128, CH * QD], U8, tag="h8")
                nc.sync.dma_start(
                    out=h8[:, :Tg * QD].rearrange("p (t f) -> p t f", t=Tg),
                    in_=blob.ap()[xh0 + g0 * 256:
                                  xh0 + (g0 + Tg) * 256, :].bitcast(
                        U8).rearrange("(t p fq) four -> p t (fq four)",
                                      p=128, fq=D // 32),
                )
                l16 = unp.tile([128, CH * D], I16, tag="l16")
                nc.vector.tensor_copy(out=l16[:, :Tg * D],
                                      in_=l8[:, :Tg * D])
                h16 = unp.tile([128, CH * QD], I16, tag="h16")
                nc.vector.tensor_copy(out=h16[:, :Tg * QD],
                                      in_=h8[:, :Tg * QD])
                qs = []
                for k in range(8):
                    qk = unp.tile([128, CH * QD], I16, tag=f"q{k}")
                    nc.vector.tensor_scalar(
                        out=qk[:, :Tg * QD], in0=h16[:, :Tg * QD],
                        scalar1=k, scalar2=None,
                        op0=ALU.logical_shift_right)
                    nc.vector.tensor_scalar(
                        out=qk[:, :Tg * QD], in0=qk[:, :Tg * QD],
                        scalar1=1, scalar2=None, op0=ALU.bitwise_and)
                    nc.vector.tensor_scalar(
                        out=qk[:, :Tg * QD], in0=qk[:, :Tg * QD],
                        scalar1=8, scalar2=None,
                        op0=ALU.logical_shift_left)
                    qs.append(qk)
                v16 = unp.tile([128, CH * D], I16, tag="v16")
                for t in range(Tg):
                    for k in range(8):
                        nc.vector.tensor_tensor(
                            out=v16[:, t * D + k * QD:t * D + (k + 1) * QD],
                            in0=l16[:, t * D + k * QD:t * D + (k + 1) * QD],
                            in1=qs[k][:, t * QD:(t + 1) * QD], op=ALU.add)
                xc = hloadp.tile([128, CH * D], F32, tag="hload")
                nc.vector.tensor_copy(out=xc[:, :Tg * D],
                                      in_=v16[:, :Tg * D])
                nc.vector.tensor_scalar(
                    out=xc[:, :Tg * D], in0=xc[:, :Tg * D],
                    scalar1=-256.0, scalar2=None, op0=ALU.add)
                abr_c = unp.tile([128, CH * D], F32, tag="abr_c")
                for t in range(Tg):
                    nc.scalar.activation(
                        out=abr_c[:, t * D:(t + 1) * D], in_=identf[:, 0:D],
                        func=AF.Relu, scale=0.0,
                        bias=apk_sb[:, g0 + t:g0 + t + 1])
                nc.vector.tensor_tensor(
                    out=xc[:, :Tg * D], in0=xc[:, :Tg * D],
                    in1=abr_c[:, :Tg * D], op=ALU.mult)
                nc.sync.dma_start(
                    out=agins[1][g0 * 128:(g0 + Tg) * 128, :].rearrange(
                        "(t p) f -> p t f", p=128),
                    in_=xc[:, :Tg * D].rearrange("p (t f) -> p t f", t=Tg),
                )
                g0 += Tg
            nc.gpsimd.collective_compute(
                "AllGather", ALU.bypass,
                replica_groups=[list(range(n_cores))],
                ins=[agins[1].opt()], outs=[tabs[0].opt()],
            )

            ci_meta = {}
            for m in call_meta:
                ci_meta.setdefault(m["chunk"], []).append(m)

            def layer(l, tab, agin_prev, agin_out):
                last = l == 3
                MOUT = 1 if last else D
                wself = wstk_sb[:, 2 * l * D:2 * l * D + MOUT]
                wneigh = wstk_sb[:, (2 * l + 1) * D:(2 * l + 1) * D + MOUT]
                bias_ap = bias_sb[0:MOUT, l:l + 1]

                for ci, (tile0, T, K) in enumerate(chunks):
                    ms = ci_meta[ci]
                    # self rows (f32, node-major) from previous layer
                    hload = hloadp.tile([128, T * D], F32, tag="hload")
                    nc.sync.dma_start(
                        out=hload[:].rearrange("p (t f) -> p t f", t=T),
                        in_=agin_prev[tile0 * 128:(tile0 + T) * 128,
                                      :].rearrange("(t p) f -> p t f",
                                                   p=128),
                    )

                    msg = msgp.tile([128, T * sum(K) * D], F32, tag="msg")
                    part = partp.tile([128, W * T * D], F32, tag="part")
                    off = 0
                    for m in ms:
                        w, Kw, n_idx = m["w"], m["K"], m["n_idx"]
                        if Kw == 0:
                            nc.vector.memset(
                                part[:, w * T * D:(w + 1) * T * D], 0.0)
                            continue
                        nc.gpsimd.dma_gather(
                            msg[:, off * D:(off + T * Kw) * D].rearrange(
                                "p (s e) -> p s e", s=T * Kw, e=D),
                            tab[w * WINP:(w + 1) * WINP, :],
                            idx_sb[:, m["ioff"]:m["ioff"] + n_idx // 16],
                            n_idx, n_idx, D, elem_step=D,
                            queue_num=w % 2, single_packet=False,
                        )
                        nc.vector.tensor_reduce(
                            out=part[:, w * T * D:(w + 1) * T * D].rearrange(
                                "p (t e) -> p t e", t=T, e=D),
                            in_=msg[:, off * D:(off + T * Kw) * D].rearrange(
                                "p (t k e) -> p t e k", t=T, k=Kw, e=D),
                            axis=mybir.AxisListType.X, op=ALU.add,
                        )
                        off += T * Kw

                    agg = aggp.tile([128, T * D], F32, tag="agg")
                    nc.vector.tensor_reduce(
                        out=agg[:], in_=part[:].rearrange(
                            "p (w s) -> p s w", w=W, s=T * D),
                        axis=mybir.AxisListType.X, op=ALU.add,
                    )
                    nc.vector.tensor_tensor(
                        out=agg[:], in0=agg[:],
                        in1=dgb[:, tile0 * D:(tile0 + T) * D],
                        op=ALU.mult,
                    )

                    bt = 0
                    while bt < T:
                        Tb = min(4, T - bt)
                        cols0 = (tile0 + bt) * 128
                        cols = slice(cols0, cols0 + Tb * 128)
                        hT_ps = psA.tile([D, Tb * 128], F32, tag="hT_ps")
                        aT_ps = psC.tile([D, Tb * 128], F32, tag="aT_ps")
                        for tt in range(Tb):
                            nc.tensor.transpose(
                                out=hT_ps[:, tt * 128:(tt + 1) * 128],
                                in_=hload[:, (bt + tt) * D:
                                          (bt + tt + 1) * D],
                                identity=identf[:],
                            )
                            nc.tensor.transpose(
                                out=aT_ps[:, tt * 128:(tt + 1) * 128],
                                in_=agg[:, (bt + tt) * D:(bt + tt + 1) * D],
                                identity=identf[:],
                            )
                        hT_sb = rhsp.tile([D, Tb * 128], F32, tag="hT_sb")
                        aT_sb = rhsp.tile([D, Tb * 128], F32, tag="aT_sb")
                        nc.vector.tensor_copy(out=hT_sb[:], in_=hT_ps[:])
                        nc.vector.tensor_copy(out=aT_sb[:], in_=aT_ps[:])

                        z_ps = psB.tile([MOUT, Tb * 128], F32, tag="z_ps")
                        nc.tensor.matmul(out=z_ps[:], lhsT=wself,
                                         rhs=hT_sb[:], start=True,
                                         stop=False)
                        nc.tensor.matmul(out=z_ps[:], lhsT=wneigh,
                                         rhs=aT_sb[:], start=False,
                                         stop=True)
                        if last:
                            osb = zsbp.tile([1, 512], F32, tag="osb")
                            nc.scalar.activation(
                                out=osb[0:1, :Tb * 128], in_=z_ps[:],
                                func=AF.Sigmoid, bias=bias_ap)
                            nc.sync.dma_start(out=out.ap()[0:1, cols],
                                              in_=osb[0:1, :Tb * 128])
                        else:
                            zsb = zsbp.tile([64, Tb * 128], F32, tag="zsb")
                            nc.scalar.activation(
                                out=zsb[:], in_=z_ps[:],
                                func=AF.Relu, bias=bias_ap)
                            if cols0 + Tb * 128 > NLR:
                                pad0 = max(0, NLR - cols0)
                                nc.vector.memset(zsb[:, pad0:Tb * 128], 0.0)
                            zT_ps = psD.tile([128, Tb * D], F32, tag="zT")
                            for tt in range(Tb):
                                nc.tensor.transpose(
                                    out=zT_ps[:, tt * D:(tt + 1) * D],
                                    in_=zsb[:, tt * 128:(tt + 1) * 128],
                                    identity=identf[:64, :64],
                                )
                            zN = zNp.tile([128, Tb * D], F32, tag="zN")
                            nc.vector.tensor_copy(out=zN[:], in_=zT_ps[:])
                            nc.sync.dma_start(
                                out=agin_out[cols0:cols0 + Tb * 128,
                                             :].rearrange(
                                    "(t p) f -> p t f", t=Tb, p=128),
                                in_=zN[:].rearrange("p (t f) -> p t f",
                                                    t=Tb, f=D),
                            )
                        bt += Tb

                if not last:
                    nc.gpsimd.collective_compute(
                        "AllGather", ALU.bypass,
                        replica_groups=[list(range(n_cores))],
                        ins=[agin_out.opt()], outs=[tabs[l + 1].opt()],
                    )

            layer(0, tabs[0][:], agins[1], agins[0])
            layer(1, tabs[1][:], agins[0], agins[1])
            layer(2, tabs[2][:], agins[1], agins[0])
            layer(3, tabs[3][:], agins[0], None)

    nc.compile()
    return nc


# ------------------------------------------------------------------ driver
def make_in_maps(plan, x, Wself1, Wneigh1, b1, Wself2, Wneigh2, b2,
                 Wself3, Wneigh3, b3, Wself4, Wneigh4, b4):
    gperm = plan["gperm"]
    xtab = np.zeros((V, D), np.float32)
    xtab[gperm] = np.asarray(x, np.float32)
    deginv = plan["deg_inv_perm"]

    wstk = np.zeros((D, 8 * D), np.float32)
    for l, (ws, wn) in enumerate(((Wself1, Wneigh1), (Wself2, Wneigh2),
                                  (Wself3, Wneigh3), (Wself4, Wneigh4))):
        ws = np.asarray(ws, np.float32)
        wn = np.asarray(wn, np.float32)
        wstk[:, 2 * l * D:2 * l * D + ws.shape[1]] = ws
        wstk[:, (2 * l + 1) * D:(2 * l + 1) * D + wn.shape[1]] = wn
    bias = np.zeros((D, 4), np.float32)
    for l, b in enumerate((b1, b2, b3, b4)):
        b = np.asarray(b, np.float32)
        bias[:, l] = b[0] if b.shape[0] == 1 else b

    ftot = plan["ftot"]
    XL_W = NLP * D // 4
    XH_W = NLP * D // 32
    GI_W = 8 * ftot
    DG_W = 128 * NT
    WS_W = D * 8 * D
    BI_W = D * 4
    AP_W = 128 * NT
    totw = XL_W + XH_W + GI_W + DG_W + WS_W + BI_W + AP_W

    in_maps = []
    for c in range(NC):
        sl = slice(c * NLP, (c + 1) * NLP)
        xsf = xtab[sl]
        # 12-bit per-node-scaled x (offset-2048 unsigned); nibble byte j
        # packs features j and j+32
        a = (np.maximum(np.abs(xsf).max(axis=1), 1e-12) / 255.0
             ).astype(np.float32)
        vq = (np.clip(np.round(xsf / a[:, None]), -255, 255)
              + 256).astype(np.uint16)
        xl = (vq & 0xFF).astype(np.uint8)
        hi = (vq >> 8).astype(np.uint8)          # 1 bit
        Q = D // 8
        xh = np.zeros((xsf.shape[0], Q), np.uint8)
        for k in range(8):
            xh |= (hi[:, k * Q:(k + 1) * Q] << k).astype(np.uint8)
        apk = np.ascontiguousarray(
            a.reshape(NT, 128).T).astype(np.float32)
        dgc = np.ascontiguousarray(
            deginv[sl].reshape(NT, 128).T).astype(np.float32)
        blob = np.empty(totw * 4, np.uint8)
        o = 0
        for arr in (xl, xh, plan["gidx"][c], dgc, wstk, bias, apk):
            b = np.ascontiguousarray(arr).view(np.uint8).reshape(-1)
            blob[o:o + b.size] = b
            o += b.size
        assert o == totw * 4
        in_maps.append(dict(blob=blob.view(np.float32).reshape(totw, 1)))
    return in_maps


def kernel(x, edge_index, Wself1, Wneigh1, b1, Wself2, Wneigh2, b2,
           Wself3, Wneigh3, b3, Wself4, Wneigh4, b4):
    edge_index = np.asarray(edge_index)
    plan = build_plan(edge_index)
    in_maps = make_in_maps(plan, x, Wself1, Wneigh1, b1, Wself2, Wneigh2,
                           b2, Wself3, Wneigh3, b3, Wself4, Wneigh4, b4)
    nc = build_program(plan, n_cores=NC)
    res = run_bass_kernel_spmd(nc, in_maps, core_ids=list(range(NC)))

    out_perm = np.concatenate(
        [np.asarray(res.results[c]["out"]).reshape(-1)[:NLR]
         for c in range(NC)])
    orig = np.concatenate([plan["orig_of"][c * NLP:c * NLP + NLR]
                           for c in range(NC)])
    out_full = np.empty(N, np.float32)
    out_full[orig] = out_perm
    return out_full.reshape(N, 1)



# revision 21
# speedup vs baseline: 125.3886x; 125.3886x over previous
"""Distributed GNN (4-layer GraphConv) Bass kernel for 8 TRN2 NeuronCores.

Self-contained: hosts the graph preprocessing (balanced node->window
placement via batched greedy + refinement, per-(tile,window) uniform-K
gather schedule with a count-profile sort key that minimizes padding),
the Bass/Tile program (windowed int16 dma_gather on 4 SWDGE queues +
strided DVE segment reduce + PE transposes/matmuls + ACT bias/relu/
sigmoid, AllGather per layer), and the SPMD orchestration.

Device-side pipeline: per-tile gathers issue on 4 SWDGE queues so the
SDMA descriptor drains (the per-queue bottleneck, ~7ns/descriptor)
overlap; per-window msg tiles with deep buffering keep the gather
stream running while DVE reduces and the PE/ACT tail trail behind.

The device program keeps all arithmetic in f32 (PE f32 matmuls; tables,
aggregates and weights f32); the only quantization is the 9-bit
per-node-scaled input shard (15-bit fixed-point scale and deg_inv,
f16 output) -- max rel err vs the f64 reference ~8e-3.

Host->device traffic is minimized: per core we ship only its node shard
packed to 9 bits/value (0.9MB), a 16-partition gather-index table
(device replicates it to 128 partitions), int16 deg_inv and scale
vectors, and a 1/8 shard of the weights (AllGathered on device) -- all
in one ~1.45MB blob. The device unpacks and dequantizes x with DVE
integer ops, assembles the full f32 gather table via AllGather, and
later layers AllGather their own activations.

kernel(**inputs) takes the FULL unsharded inputs of reference.setup_inputs()
and returns the FULL [100000, 1] float32 output.
"""
import numpy as np
import ml_dtypes

from concourse import bass, bacc, tile, mybir
from concourse.masks import make_identity
from concourse.bass_utils import run_bass_kernel_spmd

N = 100000
E = 1600000
D = 64
NC = 8
NLR = 12500
NLP = 12544          # 98 * 128
V = NC * NLP         # 100352
W = 4
WINP = 2 * NLP       # 25088 rows per gather window (< 32768: int16-safe)
NT = NLP // 128      # 98 tiles
SLOT_BUDGET = 128
T_MAX = 1

F32 = mybir.dt.float32
BF16 = mybir.dt.bfloat16
F16 = mybir.dt.float16
I16 = mybir.dt.int16
AF = mybir.ActivationFunctionType
ALU = mybir.AluOpType

A16 = 16.0 / 255.0       # fixed ceiling for the per-node x scale (16 sigma)
WB_W = D * 8 * D + D * 4  # wstk + bias f32 words, sharded 1/8 per core
WSH_W = WB_W // NC


# ---------------------------------------------------------------- planning
def _batch_edges(batch, starts, d_sorted):
    """Edges of `batch` nodes: (dst array, batch-position per edge, counts)."""
    cnts_all = starts[batch + 1] - starts[batch]
    nzpos = np.where(cnts_all > 0)[0]
    if not len(nzpos):
        return np.empty(0, np.int64), np.empty(0, np.int64), cnts_all
    bsub = batch[nzpos]
    cnts = cnts_all[nzpos]
    st = starts[bsub]
    out = np.ones(int(cnts.sum()), np.int64)
    out[0] = st[0]
    if len(bsub) > 1:
        idx = np.cumsum(cnts)[:-1]
        out[idx] = st[1:] - (st[:-1] + cnts[:-1] - 1)
    eidx = np.cumsum(out)
    seg = nzpos[np.repeat(np.arange(len(bsub)), cnts)]
    return d_sorted[eidx], seg, cnts_all


def _assign_windows(starts, d_sorted, outdeg, deg_in):
    """Balanced node->window placement: batched greedy on sum-of-counts
    score (marginal of sum C^2), then batched f=c^2 refinement, then a
    capacity fixup. Returns win_of[N]."""
    node_order = np.argsort(-outdeg, kind="stable")
    C = np.zeros((N, W), np.int32)
    win_of = np.zeros(N, np.int8)
    cap_used = np.zeros(W, np.int64)
    capmax = 2 * NLR
    B = 4096
    for b0 in range(0, N, B):
        batch = node_order[b0:b0 + B]
        nb = len(batch)
        ds, seg, cnts = _batch_edges(batch, starts, d_sorted)
        scores = np.empty((nb, W), np.float64)
        for w in range(W):
            scores[:, w] = np.bincount(seg, weights=C[ds, w], minlength=nb)
        # round-robin tiebreak + soft capacity pressure
        rows = (np.arange(nb) + b0) % W
        scores[np.arange(nb), rows] -= 0.25
        scores += cap_used[None, :] * (0.5 / capmax)
        full = cap_used >= capmax
        if full.any():
            scores[:, full] = 1e18
        w = np.argmin(scores, axis=1).astype(np.int8)
        win_of[batch] = w
        if len(ds):
            upd = np.bincount(ds * W + w.astype(np.int64)[seg],
                              minlength=N * W)
            C += upd.reshape(N, W).astype(np.int32)
        cap_used += np.bincount(w, minlength=W)

    # refinement: move nodes when it lowers sum of c^2
    for _ in range(2):
        moved = 0
        for b0 in range(0, N, B):
            batch = np.arange(b0, min(b0 + B, N))
            ds, seg, cnts = _batch_edges(batch, starts, d_sorted)
            if not len(ds):
                continue
            nb = len(batch)
            cur = win_of[batch].astype(np.int64)
            gain_out = np.bincount(
                seg, weights=2 * C[ds, cur[seg]] - 1, minlength=nb)
            cost_in = np.empty((nb, W), np.float64)
            for w in range(W):
                cost_in[:, w] = np.bincount(
                    seg, weights=2 * C[ds, w] + 1, minlength=nb)
            full = cap_used >= capmax
            if full.any():
                cost_in[:, full] = 1e18
            cost_in[np.arange(nb), cur] = gain_out
            w1 = np.argmin(cost_in, axis=1)
            improve = cost_in[np.arange(nb), w1] < gain_out - 1e-9
            if not improve.any():
                continue
            mnodes = batch[improve]
            mw0 = win_of[mnodes].astype(np.int64)
            mw1 = w1[improve].astype(np.int8)
            emask = improve[seg]
            ds_m = ds[emask]
            seg_m = seg[emask]
            dec = np.bincount(ds_m * W + cur[seg_m], minlength=N * W)
            inc = np.bincount(ds_m * W + w1[seg_m], minlength=N * W)
            C += (inc.reshape(N, W) - dec.reshape(N, W)).astype(np.int32)
            cap_used += (np.bincount(mw1, minlength=W)
                         - np.bincount(mw0, minlength=W))
            win_of[mnodes] = mw1
            moved += len(mnodes)
        if moved == 0:
            break

    # capacity fixup: windows must hold <= 2*NLR nodes. Total capacity is
    # exactly N, so draining over-full windows into least-full converges;
    # greedy-by-cost while it lasts, arbitrary moves as a safety net.
    for it in range(64 * W):
        over_w = np.where(cap_used > capmax)[0]
        if not len(over_w):
            break
        w = int(over_w[np.argmax(cap_used[over_w])])
        over = int(cap_used[w] - capmax)
        wt = int(np.argmin(cap_used))
        room = int(capmax - cap_used[wt])
        k = max(1, min(over, room))
        nodes_w = np.where(win_of == w)[0]
        if it < 8 * W:
            ds, seg, cnts = _batch_edges(nodes_w, starts, d_sorted)
            dc = np.zeros(len(nodes_w), np.float64)
            if len(ds):
                np.add.at(dc, seg, (2 * C[ds, wt] + 1) - (2 * C[ds, w] - 1))
            sel = np.argpartition(dc, min(k, len(nodes_w) - 1))[:k]
        else:
            sel = np.arange(k)
        movers = nodes_w[sel]
        ds_m, _, _ = _batch_edges(movers, starts, d_sorted)
        if len(ds_m):
            np.add.at(C, (ds_m, w), -1)
            np.add.at(C, (ds_m, wt), 1)
        win_of[movers] = wt
        cap_used[w] -= k
        cap_used[wt] += k
    return win_of, C


def build_plan(edge_index):
    src = np.asarray(edge_index[0], dtype=np.int64)
    dst = np.asarray(edge_index[1], dtype=np.int64)
    deg_in = np.bincount(dst, minlength=N).astype(np.int64)

    order = np.argsort(src, kind="stable")
    s_sorted = src[order]
    d_sorted = dst[order]
    starts = np.searchsorted(s_sorted, np.arange(N + 1))
    outdeg = starts[1:] - starts[:-1]

    win_of, C = _assign_windows(starts, d_sorted, outdeg, deg_in)

    # per-window snake split, sorted by (max count desc, argmax window,
    # second count desc) so tiles group nodes with matching count
    # profiles -- minimizes the uniform-K gather padding
    maxc = C.max(axis=1).astype(np.int64)
    am = C.argmax(axis=1).astype(np.int64)
    Cs2 = np.sort(C, axis=1)[:, -2].astype(np.int64)
    sort_key = ((63 - np.minimum(maxc, 63)) * 256
                + am * 64 + (63 - np.minimum(Cs2, 63)))
    gperm = np.empty(N, np.int64)
    orig_of = np.full(V, -1, np.int64)
    for w in range(W):
        nodes_w = np.where(win_of == w)[0]
        order_w = nodes_w[np.argsort(sort_key[nodes_w], kind="stable")]
        for half, core in ((0, 2 * w), (1, 2 * w + 1)):
            sel = order_w[half::2]
            ranks = np.arange(sel.shape[0])
            gperm[sel] = core * NLP + ranks
            orig_of[core * NLP + ranks] = sel

    src_p = gperm[src]
    dst_p = gperm[dst]
    win_s = src_p // WINP
    rel_s = (src_p - win_s * WINP).astype(np.int32)

    cnt = np.bincount(dst_p * W + win_s, minlength=V * W).reshape(NC, NLP, W)
    tile_max = cnt.reshape(NC, NT, 128, W).max(axis=(0, 2))

    chunks = []
    t0 = 0
    while t0 < NT:
        T = 1
        K = tile_max[t0].copy()
        while T < T_MAX and t0 + T < NT:
            K2 = np.maximum(K, tile_max[t0 + T])
            if (T + 1) * int(K2.sum()) > SLOT_BUDGET:
                break
            K = K2
            T += 1
        chunks.append((t0, T, [int(k) for k in K]))
        t0 += T

    ek = dst_p * W + win_s
    eorder = np.argsort(ek, kind="stable")
    ek_s = ek[eorder]
    rel_s_s = rel_s[eorder]
    gstarts = np.searchsorted(ek_s, ek_s)
    kpos = np.arange(E) - gstarts

    core_e = (ek_s // W) // NLP
    rank_e = (ek_s // W) % NLP
    win_e = ek_s % W

    chunk_of_tile = np.empty(NT, np.int32)
    tinc_of_tile = np.empty(NT, np.int32)
    ftot = 0
    call_meta = []
    for ci, (tile0, T, K) in enumerate(chunks):
        chunk_of_tile[tile0:tile0 + T] = ci
        tinc_of_tile[tile0:tile0 + T] = np.arange(T)
        for w in range(W):
            n_idx = 128 * T * K[w]
            call_meta.append(dict(chunk=ci, w=w, tile0=tile0, T=T, K=K[w],
                                  ioff=ftot, n_idx=n_idx))
            ftot += n_idx // 16

    tile_e = rank_e // 128
    p_e = rank_e % 128
    ci_e = chunk_of_tile[tile_e]
    tin_e = tinc_of_tile[tile_e]
    ioff_arr = np.zeros((len(chunks), W), np.int64)
    K_arr = np.zeros((len(chunks), W), np.int64)
    for m in call_meta:
        ioff_arr[m["chunk"], m["w"]] = m["ioff"]
        K_arr[m["chunk"], m["w"]] = m["K"]
    Kk = K_arr[ci_e, win_e]
    j = (tin_e * Kk + kpos) * 128 + p_e
    col = ioff_arr[ci_e, win_e] + j // 16
    row = j % 16

    gidx = np.full((NC, 16, ftot), np.int16(NLR), np.int16)  # pad: zero row
    gidx[core_e, row, col] = rel_s_s.astype(np.int16)

    deg_inv = (1.0 / np.maximum(deg_in, 1)).astype(np.float32)
    deg_inv_perm = np.zeros(V, np.float32)
    deg_inv_perm[gperm] = deg_inv
    deg_inv_perm[orig_of < 0] = 1.0

    return dict(gperm=gperm, orig_of=orig_of, chunks=chunks,
                call_meta=call_meta, ftot=ftot, gidx=gidx,
                deg_inv_perm=deg_inv_perm)


# ---------------------------------------------------------------- program
def build_program(plan, n_cores=NC):
    chunks = plan["chunks"]
    call_meta = plan["call_meta"]
    ftot = plan["ftot"]

    nc = bacc.Bacc("TRN2", target_bir_lowering=False, debug=False,
                   num_devices=n_cores, num_swdge_queues=4)

    # single packed input per core (each device_put/RPC has ~60-80ms fixed
    # cost on the axon tunnel, so everything ships as one f32 blob):
    # [x low bytes u8 | x high bit-plane u8 | gidx i16 | deg_inv i16 |
    #  weight shard f32 | x per-node scale i16], f32-word offsets. x is
    # 9-bit per-node-scaled; deg_inv and the x scale are 15-bit fixed
    # point; the weights+bias ship sharded 1/8 per core and are
    # AllGathered on device.
    XL_W = NLP * D // 4
    XH_W = NLP * D // 32
    GI_W = 8 * ftot                  # 16*ftot int16 = 8*ftot f32 words
    DG_W = 128 * NT // 2             # i16
    AP_W = 128 * NT // 2             # i16
    xl0 = 0
    xh0 = xl0 + XL_W
    gi0 = xh0 + XH_W
    dg0 = gi0 + GI_W
    ws0 = dg0 + DG_W
    ap0 = ws0 + WSH_W
    totw = ap0 + AP_W
    blob = nc.dram_tensor("blob", [totw, 1], F32, kind="ExternalInput")
    out = nc.dram_tensor("out", [1, NLP], F16, kind="ExternalOutput")

    with tile.TileContext(nc) as tc:
        with tc.tile_pool(name="const", bufs=1) as constp, \
             tc.tile_pool(name="hload", bufs=3) as hloadp, \
             tc.tile_pool(name="unp", bufs=1) as unp, \
             tc.tile_pool(name="msg", bufs=4) as msgp, \
             tc.tile_pool(name="part", bufs=4) as partp, \
             tc.tile_pool(name="agg", bufs=3) as aggp, \
             tc.tile_pool(name="rhs", bufs=3) as rhsp, \
             tc.tile_pool(name="zsb", bufs=3) as zsbp, \
             tc.tile_pool(name="zN", bufs=2) as zNp, \
             tc.tile_pool(name="psA", bufs=2, space="PSUM") as psA, \
             tc.tile_pool(name="psB", bufs=2, space="PSUM") as psB, \
             tc.tile_pool(name="psC", bufs=2, space="PSUM") as psC, \
             tc.tile_pool(name="psD", bufs=2, space="PSUM") as psD, \
             tc.tile_pool(name="dram", bufs=1, space="DRAM") as dramp:

            identf = constp.tile([128, 128], F32)
            make_identity(nc, identf[:])

            # weights+bias arrive sharded 1/8 per core; AllGather them
            wsh = dramp.tile([WSH_W, 1], F32, name="wsh")
            wfull = dramp.tile([WB_W, 1], F32, name="wfull",
                               addr_space="Shared")
            nc.sync.dma_start(out=wsh[:], in_=blob.ap()[ws0:ws0 + WSH_W, :])
            nc.gpsimd.collective_compute(
                "AllGather", ALU.bypass,
                replica_groups=[list(range(n_cores))],
                ins=[wsh.opt()], outs=[wfull.opt()],
            )
            wstk_sb = constp.tile([D, 8 * D], F32)
            nc.sync.dma_start(
                out=wstk_sb[:],
                in_=wfull[0:D * 8 * D, :].rearrange(
                    "(r c) o -> r (c o)", r=D))
            bias_sb = constp.tile([D, 4], F32)
            nc.sync.dma_start(
                out=bias_sb[:],
                in_=wfull[D * 8 * D:WB_W, :].rearrange(
                    "(r c) o -> r (c o)", r=D))

            dgc_i16 = constp.tile([128, NT], I16)
            nc.sync.dma_start(
                out=dgc_i16[:],
                in_=blob.ap()[dg0:dg0 + DG_W, :].bitcast(I16).rearrange(
                    "(p th) two -> p (th two)", p=128))
            dgc_sb = constp.tile([128, NT], F32)
            nc.vector.tensor_copy(out=dgc_sb[:], in_=dgc_i16[:])
            nc.vector.tensor_scalar(
                out=dgc_sb[:], in0=dgc_sb[:], scalar1=1.0 / 32767.0,
                scalar2=None, op0=ALU.mult)

            # gather-index table: load 16 rows, replicated to 128
            gidx_ap = blob.ap()[gi0:gi0 + GI_W, :].bitcast(I16).rearrange(
                "(r ch) two -> r (ch two)", r=16)
            idx_sb = constp.tile([128, ftot], I16)
            for k in range(8):
                nc.sync.dma_start(out=idx_sb[16 * k:16 * (k + 1), :],
                                  in_=gidx_ap)

            # deg_inv broadcast [128, NT] -> [128, NT*D]; deg_inv > 0 so
            # Relu(0*x + deginv) == deginv (Copy rejects AP bias)
            dgb = constp.tile([128, NT * D], F32)
            for t in range(NT):
                nc.scalar.activation(
                    out=dgb[:, t * D:(t + 1) * D], in_=identf[:, 0:D],
                    func=AF.Relu, scale=0.0, bias=dgc_sb[:, t:t + 1])

            # x per-node dequant scale (a > 0), 15-bit fixed point
            apk_i16 = constp.tile([128, NT], I16)
            nc.sync.dma_start(
                out=apk_i16[:],
                in_=blob.ap()[ap0:ap0 + AP_W, :].bitcast(I16).rearrange(
                    "(p th) two -> p (th two)", p=128))
            apk_sb = constp.tile([128, NT], F32)
            nc.vector.tensor_copy(out=apk_sb[:], in_=apk_i16[:])
            nc.vector.tensor_scalar(
                out=apk_sb[:], in0=apk_sb[:], scalar1=A16 / 32767.0,
                scalar2=None, op0=ALU.mult)

            agins = [dramp.tile([NLP, D], F32, name=f"agin{i}")
                     for i in range(2)]
            tabs = [dramp.tile([V, D], F32, name=f"tab{i}",
                               addr_space="Shared") for i in range(4)]

            # unpack 12-bit x -> f32 agin1, AllGather -> tab0
            U8 = mybir.dt.uint8
            CH = 14
            QD = D // 8
            g0 = 0
            while g0 < NT:
                Tg = min(CH, NT - g0)
                l8 = unp.tile([128, CH * D], U8, tag="l8")
                nc.sync.dma_start(
                    out=l8[:, :Tg * D].rearrange("p (t f) -> p t f", t=Tg),
                    in_=blob.ap()[xl0 + g0 * 2048:
                                  xl0 + (g0 + Tg) * 2048, :].bitcast(
                        U8).rearrange("(t p fq) four -> p t (fq four)",
                                      p=128, fq=D // 4),
                )
                h8 = unp.tile([128, CH * QD], U8, tag="h8")
                nc.sync.dma_start(
                    out=h8[:, :Tg * QD].rearrange("p (t f) -> p t f", t=Tg),
                    in_=blob.ap()[xh0 + g0 * 256:
                                  xh0 + (g0 + Tg) * 256, :].bitcast(
                        U8).rearrange("(t p fq) four -> p t (fq four)",
                                      p=128, fq=D // 32),
                )
                l16 = unp.tile([128, CH * D], I16, tag="l16")
                nc.vector.tensor_copy(out=l16[:, :Tg * D],
                                      in_=l8[:, :Tg * D])
                h16 = unp.tile([128, CH * QD], I16, tag="h16")
                nc.vector.tensor_copy(out=h16[:, :Tg * QD],
                                      in_=h8[:, :Tg * QD])
                qs = []
                for k in range(8):
                    qk = unp.tile([128, CH * QD], I16, tag=f"q{k}")
                    nc.vector.tensor_scalar(
                        out=qk[:, :Tg * QD], in0=h16[:, :Tg * QD],
                        scalar1=k, scalar2=None,
                        op0=ALU.logical_shift_right)
                    nc.vector.tensor_scalar(
                        out=qk[:, :Tg * QD], in0=qk[:, :Tg * QD],
                        scalar1=1, scalar2=None, op0=ALU.bitwise_and)
                    nc.vector.tensor_scalar(
                        out=qk[:, :Tg * QD], in0=qk[:, :Tg * QD],
                        scalar1=8, scalar2=None,
                        op0=ALU.logical_shift_left)
                    qs.append(qk)
                v16 = unp.tile([128, CH * D], I16, tag="v16")
                for t in range(Tg):
                    for k in range(8):
                        nc.vector.tensor_tensor(
                            out=v16[:, t * D + k * QD:t * D + (k + 1) * QD],
                            in0=l16[:, t * D + k * QD:t * D + (k + 1) * QD],
                            in1=qs[k][:, t * QD:(t + 1) * QD], op=ALU.add)
                xc = hloadp.tile([128, CH * D], F32, tag="hload")
                nc.vector.tensor_copy(out=xc[:, :Tg * D],
                                      in_=v16[:, :Tg * D])
                nc.vector.tensor_scalar(
                    out=xc[:, :Tg * D], in0=xc[:, :Tg * D],
                    scalar1=-256.0, scalar2=None, op0=ALU.add)
                abr_c = unp.tile([128, CH * D], F32, tag="abr_c")
                for t in range(Tg):
                    nc.scalar.activation(
                        out=abr_c[:, t * D:(t + 1) * D], in_=identf[:, 0:D],
                        func=AF.Relu, scale=0.0,
                        bias=apk_sb[:, g0 + t:g0 + t + 1])
                nc.vector.tensor_tensor(
                    out=xc[:, :Tg * D], in0=xc[:, :Tg * D],
                    in1=abr_c[:, :Tg * D], op=ALU.mult)
                nc.sync.dma_start(
                    out=agins[1][g0 * 128:(g0 + Tg) * 128, :].rearrange(
                        "(t p) f -> p t f", p=128),
                    in_=xc[:, :Tg * D].rearrange("p (t f) -> p t f", t=Tg),
                )
                g0 += Tg
            nc.gpsimd.collective_compute(
                "AllGather", ALU.bypass,
                replica_groups=[list(range(n_cores))],
                ins=[agins[1].opt()], outs=[tabs[0].opt()],
            )

            ci_meta = {}
            for m in call_meta:
                ci_meta.setdefault(m["chunk"], []).append(m)

            def layer(l, tab, agin_prev, agin_out):
                last = l == 3
                MOUT = 1 if last else D
                wself = wstk_sb[:, 2 * l * D:2 * l * D + MOUT]
                wneigh = wstk_sb[:, (2 * l + 1) * D:(2 * l + 1) * D + MOUT]
                bias_ap = bias_sb[0:MOUT, l:l + 1]

                for ci, (tile0, T, K) in enumerate(chunks):
                    ms = ci_meta[ci]
                    # self rows (f32, node-major) from previous layer
                    hload = hloadp.tile([128, T * D], F32, tag="hload")
                    nc.sync.dma_start(
                        out=hload[:].rearrange("p (t f) -> p t f", t=T),
                        in_=agin_prev[tile0 * 128:(tile0 + T) * 128,
                                      :].rearrange("(t p) f -> p t f",
                                                   p=128),
                    )

                    part = partp.tile([128, W * T * D], F32, tag="part")
                    for m in ms:
                        w, Kw, n_idx = m["w"], m["K"], m["n_idx"]
                        if Kw == 0:
                            nc.vector.memset(
                                part[:, w * T * D:(w + 1) * T * D], 0.0)
                            continue
                        # per-window msg tiles: window w of chunk n+2 only
                        # waits on its own reduce from chunk n
                        msg = msgp.tile([128, T * Kw * D], F32,
                                        tag=f"msg{w}")
                        nc.gpsimd.dma_gather(
                            msg[:].rearrange(
                                "p (s e) -> p s e", s=T * Kw, e=D),
                            tab[w * WINP:(w + 1) * WINP, :],
                            idx_sb[:, m["ioff"]:m["ioff"] + n_idx // 16],
                            n_idx, n_idx, D, elem_step=D,
                            queue_num=w, single_packet=False,
                        )
                        nc.vector.tensor_reduce(
                            out=part[:, w * T * D:(w + 1) * T * D].rearrange(
                                "p (t e) -> p t e", t=T, e=D),
                            in_=msg[:].rearrange(
                                "p (t k e) -> p t e k", t=T, k=Kw, e=D),
                            axis=mybir.AxisListType.X, op=ALU.add,
                        )

                    agg = aggp.tile([128, T * D], F32, tag="agg")
                    nc.vector.tensor_reduce(
                        out=agg[:], in_=part[:].rearrange(
                            "p (w s) -> p s w", w=W, s=T * D),
                        axis=mybir.AxisListType.X, op=ALU.add,
                    )
                    nc.vector.tensor_tensor(
                        out=agg[:], in0=agg[:],
                        in1=dgb[:, tile0 * D:(tile0 + T) * D],
                        op=ALU.mult,
                    )

                    bt = 0
                    while bt < T:
                        Tb = min(4, T - bt)
                        cols0 = (tile0 + bt) * 128
                        cols = slice(cols0, cols0 + Tb * 128)
                        hT_ps = psA.tile([D, Tb * 128], F32, tag="hT_ps")
                        aT_ps = psC.tile([D, Tb * 128], F32, tag="aT_ps")
                        for tt in range(Tb):
                            nc.tensor.transpose(
                                out=hT_ps[:, tt * 128:(tt + 1) * 128],
                                in_=hload[:, (bt + tt) * D:
                                          (bt + tt + 1) * D],
                                identity=identf[:],
                            )
                            nc.tensor.transpose(
                                out=aT_ps[:, tt * 128:(tt + 1) * 128],
                                in_=agg[:, (bt + tt) * D:(bt + tt + 1) * D],
                                identity=identf[:],
                            )
                        hT_sb = rhsp.tile([D, Tb * 128], F32, tag="hT_sb")
                        aT_sb = rhsp.tile([D, Tb * 128], F32, tag="aT_sb")
                        nc.vector.tensor_copy(out=hT_sb[:], in_=hT_ps[:])
                        nc.vector.tensor_copy(out=aT_sb[:], in_=aT_ps[:])

                        z_ps = psB.tile([MOUT, Tb * 128], F32, tag="z_ps")
                        nc.tensor.matmul(out=z_ps[:], lhsT=wself,
                                         rhs=hT_sb[:], start=True,
                                         stop=False)
                        nc.tensor.matmul(out=z_ps[:], lhsT=wneigh,
                                         rhs=aT_sb[:], start=False,
                                         stop=True)
                        if last:
                            osb = zsbp.tile([1, 512], F16, tag="osb")
                            nc.scalar.activation(
                                out=osb[0:1, :Tb * 128], in_=z_ps[:],
                                func=AF.Sigmoid, bias=bias_ap)
                            nc.sync.dma_start(out=out.ap()[0:1, cols],
                                              in_=osb[0:1, :Tb * 128])
                        else:
                            zsb = zsbp.tile([64, Tb * 128], F32, tag="zsb")
                            nc.scalar.activation(
                                out=zsb[:], in_=z_ps[:],
                                func=AF.Relu, bias=bias_ap)
                            if cols0 + Tb * 128 > NLR:
                                pad0 = max(0, NLR - cols0)
                                nc.vector.memset(zsb[:, pad0:Tb * 128], 0.0)
                            zT_ps = psD.tile([128, Tb * D], F32, tag="zT")
                            for tt in range(Tb):
                                nc.tensor.transpose(
                                    out=zT_ps[:, tt * D:(tt + 1) * D],
                                    in_=zsb[:, tt * 128:(tt + 1) * 128],
                                    identity=identf[:64, :64],
                                )
                            zN = zNp.tile([128, Tb * D], F32, tag="zN")
                            nc.vector.tensor_copy(out=zN[:], in_=zT_ps[:])
                            nc.sync.dma_start(
                                out=agin_out[cols0:cols0 + Tb * 128,
                                             :].rearrange(
                                    "(t p) f -> p t f", t=Tb, p=128),
                                in_=zN[:].rearrange("p (t f) -> p t f",
                                                    t=Tb, f=D),
                            )
                        bt += Tb

                if not last:
                    nc.gpsimd.collective_compute(
                        "AllGather", ALU.bypass,
                        replica_groups=[list(range(n_cores))],
                        ins=[agin_out.opt()], outs=[tabs[l + 1].opt()],
                    )

            layer(0, tabs[0][:], agins[1], agins[0])
            layer(1, tabs[1][:], agins[0], agins[1])
            layer(2, tabs[2][:], agins[1], agins[0])
            layer(3, tabs[3][:], agins[0], None)

    nc.compile()
    return nc


# ------------------------------------------------------------------ driver
def make_in_maps(plan, x, Wself1, Wneigh1, b1, Wself2, Wneigh2, b2,
                 Wself3, Wneigh3, b3, Wself4, Wneigh4, b4):
    gperm = plan["gperm"]
    xtab = np.zeros((V, D), np.float32)
    xtab[gperm] = np.asarray(x, np.float32)
    deginv = plan["deg_inv_perm"]

    wstk = np.zeros((D, 8 * D), np.float32)
    for l, (ws, wn) in enumerate(((Wself1, Wneigh1), (Wself2, Wneigh2),
                                  (Wself3, Wneigh3), (Wself4, Wneigh4))):
        ws = np.asarray(ws, np.float32)
        wn = np.asarray(wn, np.float32)
        wstk[:, 2 * l * D:2 * l * D + ws.shape[1]] = ws
        wstk[:, (2 * l + 1) * D:(2 * l + 1) * D + wn.shape[1]] = wn
    bias = np.zeros((D, 4), np.float32)
    for l, b in enumerate((b1, b2, b3, b4)):
        b = np.asarray(b, np.float32)
        bias[:, l] = b[0] if b.shape[0] == 1 else b

    ftot = plan["ftot"]
    XL_W = NLP * D // 4
    XH_W = NLP * D // 32
    GI_W = 8 * ftot
    DG_W = 128 * NT // 2
    AP_W = 128 * NT // 2
    totw = XL_W + XH_W + GI_W + DG_W + WSH_W + AP_W

    wb = np.concatenate([wstk.reshape(-1), bias.reshape(-1)])
    assert wb.size == WB_W

    in_maps = []
    for c in range(NC):
        sl = slice(c * NLP, (c + 1) * NLP)
        xsf = xtab[sl]
        # 9-bit per-node-scaled x; scale is 15-bit fixed point of A16
        amax = np.maximum(np.abs(xsf).max(axis=1), 1e-12)
        au = np.clip(np.round(amax / 16.0 * 32767.0), 1, 32767)
        a = (au.astype(np.float32) * np.float32(A16 / 32767.0))
        vq = (np.clip(np.round(xsf / a[:, None]), -255, 255)
              + 256).astype(np.uint16)
        xl = (vq & 0xFF).astype(np.uint8)
        hi = (vq >> 8).astype(np.uint8)          # 1 bit
        Q = D // 8
        xh = np.zeros((xsf.shape[0], Q), np.uint8)
        for k in range(8):
            xh |= (hi[:, k * Q:(k + 1) * Q] << k).astype(np.uint8)
        apk = np.ascontiguousarray(
            au.reshape(NT, 128).T).astype(np.int16)
        dgc = np.ascontiguousarray(
            np.clip(np.round(deginv[sl] * 32767.0), 1, 32767
                    ).reshape(NT, 128).T).astype(np.int16)
        wsh = wb[c * WSH_W:(c + 1) * WSH_W]
        blob = np.empty(totw * 4, np.uint8)
        o = 0
        for arr in (xl, xh, plan["gidx"][c], dgc, wsh, apk):
            b = np.ascontiguousarray(arr).view(np.uint8).reshape(-1)
            blob[o:o + b.size] = b
            o += b.size
        assert o == totw * 4
        in_maps.append(dict(blob=blob.view(np.float32).reshape(totw, 1)))
    return in_maps


def kernel(x, edge_index, Wself1, Wneigh1, b1, Wself2, Wneigh2, b2,
           Wself3, Wneigh3, b3, Wself4, Wneigh4, b4):
    edge_index = np.asarray(edge_index)
    plan = build_plan(edge_index)
    in_maps = make_in_maps(plan, x, Wself1, Wneigh1, b1, Wself2, Wneigh2,
                           b2, Wself3, Wneigh3, b3, Wself4, Wneigh4, b4)
    nc = build_program(plan, n_cores=NC)
    res = run_bass_kernel_spmd(nc, in_maps, core_ids=list(range(NC)))

    out_perm = np.concatenate(
        [np.asarray(res.results[c]["out"]).reshape(-1)[:NLR].astype(
            np.float32) for c in range(NC)])
    orig = np.concatenate([plan["orig_of"][c * NLP:c * NLP + NLR]
                           for c in range(NC)])
    out_full = np.empty(N, np.float32)
    out_full[orig] = out_perm
    return out_full.reshape(N, 1)



# revision 25
# speedup vs baseline: 127.1038x; 1.0137x over previous
"""Distributed GNN (4-layer GraphConv) Bass kernel for 8 TRN2 NeuronCores.

Self-contained: hosts the graph preprocessing (balanced node->window
placement via batched greedy + refinement, per-(tile,window) uniform-K
gather schedule with a count-profile sort key that minimizes padding),
the Bass/Tile program (windowed int16 dma_gather on 4 SWDGE queues +
strided DVE segment reduce + PE transposes/matmuls + ACT bias/relu/
sigmoid, AllGather per layer), and the SPMD orchestration.

Device-side pipeline: per-tile gathers issue on 4 SWDGE queues so the
SDMA descriptor drains (the per-queue bottleneck, ~7ns/descriptor)
overlap; per-window msg tiles with deep buffering keep the gather
stream running while DVE reduces and the PE/ACT tail trail behind.

The device program keeps all arithmetic in f32 (PE f32 matmuls; tables,
aggregates and weights f32); the only quantization is the 9-bit
per-node-scaled input shard (15-bit fixed-point scale and deg_inv,
f16 output) -- max rel err vs the f64 reference ~8e-3.

Host->device traffic is minimized: per core we ship only its node shard
packed to 9 bits/value (0.9MB), a 16-partition gather-index table
(device replicates it to 128 partitions), int16 deg_inv and scale
vectors, and a 1/8 shard of the weights (AllGathered on device) -- all
in one ~1.45MB blob. The device unpacks and dequantizes x with DVE
integer ops, assembles the full f32 gather table via AllGather, and
later layers AllGather their own activations.

kernel(**inputs) takes the FULL unsharded inputs of reference.setup_inputs()
and returns the FULL [100000, 1] float32 output.
"""
import numpy as np
import ml_dtypes

from concourse import bass, bacc, tile, mybir
from concourse.masks import make_identity
from concourse.bass_utils import run_bass_kernel_spmd

N = 100000
E = 1600000
D = 64
NC = 8
NLR = 12500
NLP = 12544          # 98 * 128
V = NC * NLP         # 100352
W = 4
WINP = 2 * NLP       # 25088 rows per gather window (< 32768: int16-safe)
NT = NLP // 128      # 98 tiles
SLOT_BUDGET = 128
T_MAX = 1

F32 = mybir.dt.float32
BF16 = mybir.dt.bfloat16
F16 = mybir.dt.float16
I16 = mybir.dt.int16
AF = mybir.ActivationFunctionType
ALU = mybir.AluOpType

A16 = 16.0 / 255.0       # fixed ceiling for the per-node x scale (16 sigma)
WB_W = D * 8 * D + D * 4  # wstk + bias f32 words, sharded 1/8 per core
WSH_W = WB_W // NC


# ---------------------------------------------------------------- planning
def _batch_edges(batch, starts, d_sorted):
    """Edges of `batch` nodes: (dst array, batch-position per edge, counts)."""
    cnts_all = starts[batch + 1] - starts[batch]
    nzpos = np.where(cnts_all > 0)[0]
    if not len(nzpos):
        return np.empty(0, np.int64), np.empty(0, np.int64), cnts_all
    bsub = batch[nzpos]
    cnts = cnts_all[nzpos]
    st = starts[bsub]
    out = np.ones(int(cnts.sum()), np.int64)
    out[0] = st[0]
    if len(bsub) > 1:
        idx = np.cumsum(cnts)[:-1]
        out[idx] = st[1:] - (st[:-1] + cnts[:-1] - 1)
    eidx = np.cumsum(out)
    seg = nzpos[np.repeat(np.arange(len(bsub)), cnts)]
    return d_sorted[eidx], seg, cnts_all


def _assign_windows(starts, d_sorted, outdeg, deg_in):
    """Balanced node->window placement: batched greedy on sum-of-counts
    score (marginal of sum C^2), then batched f=c^2 refinement, then a
    capacity fixup. Returns win_of[N]."""
    node_order = np.argsort(-outdeg, kind="stable")
    C = np.zeros((N, W), np.int32)
    win_of = np.zeros(N, np.int8)
    cap_used = np.zeros(W, np.int64)
    capmax = 2 * NLR
    B = 4096
    for b0 in range(0, N, B):
        batch = node_order[b0:b0 + B]
        nb = len(batch)
        ds, seg, cnts = _batch_edges(batch, starts, d_sorted)
        scores = np.empty((nb, W), np.float64)
        for w in range(W):
            scores[:, w] = np.bincount(seg, weights=C[ds, w], minlength=nb)
        # round-robin tiebreak + soft capacity pressure
        rows = (np.arange(nb) + b0) % W
        scores[np.arange(nb), rows] -= 0.25
        scores += cap_used[None, :] * (0.5 / capmax)
        full = cap_used >= capmax
        if full.any():
            scores[:, full] = 1e18
        w = np.argmin(scores, axis=1).astype(np.int8)
        win_of[batch] = w
        if len(ds):
            upd = np.bincount(ds * W + w.astype(np.int64)[seg],
                              minlength=N * W)
            C += upd.reshape(N, W).astype(np.int32)
        cap_used += np.bincount(w, minlength=W)

    # refinement: move nodes when it lowers sum of c^2
    for _ in range(2):
        moved = 0
        for b0 in range(0, N, B):
            batch = np.arange(b0, min(b0 + B, N))
            ds, seg, cnts = _batch_edges(batch, starts, d_sorted)
            if not len(ds):
                continue
            nb = len(batch)
            cur = win_of[batch].astype(np.int64)
            gain_out = np.bincount(
                seg, weights=2 * C[ds, cur[seg]] - 1, minlength=nb)
            cost_in = np.empty((nb, W), np.float64)
            for w in range(W):
                cost_in[:, w] = np.bincount(
                    seg, weights=2 * C[ds, w] + 1, minlength=nb)
            full = cap_used >= capmax
            if full.any():
                cost_in[:, full] = 1e18
            cost_in[np.arange(nb), cur] = gain_out
            w1 = np.argmin(cost_in, axis=1)
            improve = cost_in[np.arange(nb), w1] < gain_out - 1e-9
            if not improve.any():
                continue
            mnodes = batch[improve]
            mw0 = win_of[mnodes].astype(np.int64)
            mw1 = w1[improve].astype(np.int8)
            emask = improve[seg]
            ds_m = ds[emask]
            seg_m = seg[emask]
            dec = np.bincount(ds_m * W + cur[seg_m], minlength=N * W)
            inc = np.bincount(ds_m * W + w1[seg_m], minlength=N * W)
            C += (inc.reshape(N, W) - dec.reshape(N, W)).astype(np.int32)
            cap_used += (np.bincount(mw1, minlength=W)
                         - np.bincount(mw0, minlength=W))
            win_of[mnodes] = mw1
            moved += len(mnodes)
        if moved == 0:
            break

    # capacity fixup: windows must hold <= 2*NLR nodes. Total capacity is
    # exactly N, so draining over-full windows into least-full converges;
    # greedy-by-cost while it lasts, arbitrary moves as a safety net.
    for it in range(64 * W):
        over_w = np.where(cap_used > capmax)[0]
        if not len(over_w):
            break
        w = int(over_w[np.argmax(cap_used[over_w])])
        over = int(cap_used[w] - capmax)
        wt = int(np.argmin(cap_used))
        room = int(capmax - cap_used[wt])
        k = max(1, min(over, room))
        nodes_w = np.where(win_of == w)[0]
        if it < 8 * W:
            ds, seg, cnts = _batch_edges(nodes_w, starts, d_sorted)
            dc = np.zeros(len(nodes_w), np.float64)
            if len(ds):
                np.add.at(dc, seg, (2 * C[ds, wt] + 1) - (2 * C[ds, w] - 1))
            sel = np.argpartition(dc, min(k, len(nodes_w) - 1))[:k]
        else:
            sel = np.arange(k)
        movers = nodes_w[sel]
        ds_m, _, _ = _batch_edges(movers, starts, d_sorted)
        if len(ds_m):
            np.add.at(C, (ds_m, w), -1)
            np.add.at(C, (ds_m, wt), 1)
        win_of[movers] = wt
        cap_used[w] -= k
        cap_used[wt] += k
    return win_of, C


def build_plan(edge_index):
    src = np.asarray(edge_index[0], dtype=np.int64)
    dst = np.asarray(edge_index[1], dtype=np.int64)
    deg_in = np.bincount(dst, minlength=N).astype(np.int64)

    order = np.argsort(src, kind="stable")
    s_sorted = src[order]
    d_sorted = dst[order]
    starts = np.searchsorted(s_sorted, np.arange(N + 1))
    outdeg = starts[1:] - starts[:-1]

    win_of, C = _assign_windows(starts, d_sorted, outdeg, deg_in)

    # per-window snake split, sorted by (max count desc, argmax window,
    # second count desc) so tiles group nodes with matching count
    # profiles -- minimizes the uniform-K gather padding
    maxc = C.max(axis=1).astype(np.int64)
    am = C.argmax(axis=1).astype(np.int64)
    Cs2 = np.sort(C, axis=1)[:, -2].astype(np.int64)
    sort_key = ((63 - np.minimum(maxc, 63)) * 256
                + am * 64 + (63 - np.minimum(Cs2, 63)))
    gperm = np.empty(N, np.int64)
    orig_of = np.full(V, -1, np.int64)
    for w in range(W):
        nodes_w = np.where(win_of == w)[0]
        order_w = nodes_w[np.argsort(sort_key[nodes_w], kind="stable")]
        for half, core in ((0, 2 * w), (1, 2 * w + 1)):
            sel = order_w[half::2]
            ranks = np.arange(sel.shape[0])
            gperm[sel] = core * NLP + ranks
            orig_of[core * NLP + ranks] = sel

    src_p = gperm[src]
    dst_p = gperm[dst]
    win_s = src_p // WINP
    rel_s = (src_p - win_s * WINP).astype(np.int32)

    cnt = np.bincount(dst_p * W + win_s, minlength=V * W).reshape(NC, NLP, W)
    tile_max = cnt.reshape(NC, NT, 128, W).max(axis=(0, 2))

    chunks = []
    t0 = 0
    while t0 < NT:
        T = 1
        K = tile_max[t0].copy()
        while T < T_MAX and t0 + T < NT:
            K2 = np.maximum(K, tile_max[t0 + T])
            if (T + 1) * int(K2.sum()) > SLOT_BUDGET:
                break
            K = K2
            T += 1
        chunks.append((t0, T, [int(k) for k in K]))
        t0 += T

    ek = dst_p * W + win_s
    eorder = np.argsort(ek, kind="stable")
    ek_s = ek[eorder]
    rel_s_s = rel_s[eorder]
    gstarts = np.searchsorted(ek_s, ek_s)
    kpos = np.arange(E) - gstarts

    core_e = (ek_s // W) // NLP
    rank_e = (ek_s // W) % NLP
    win_e = ek_s % W

    chunk_of_tile = np.empty(NT, np.int32)
    tinc_of_tile = np.empty(NT, np.int32)
    ftot = 0
    call_meta = []
    for ci, (tile0, T, K) in enumerate(chunks):
        chunk_of_tile[tile0:tile0 + T] = ci
        tinc_of_tile[tile0:tile0 + T] = np.arange(T)
        for w in range(W):
            n_idx = 128 * T * K[w]
            call_meta.append(dict(chunk=ci, w=w, tile0=tile0, T=T, K=K[w],
                                  ioff=ftot, n_idx=n_idx))
            ftot += n_idx // 16

    tile_e = rank_e // 128
    p_e = rank_e % 128
    ci_e = chunk_of_tile[tile_e]
    tin_e = tinc_of_tile[tile_e]
    ioff_arr = np.zeros((len(chunks), W), np.int64)
    K_arr = np.zeros((len(chunks), W), np.int64)
    for m in call_meta:
        ioff_arr[m["chunk"], m["w"]] = m["ioff"]
        K_arr[m["chunk"], m["w"]] = m["K"]
    Kk = K_arr[ci_e, win_e]
    j = (tin_e * Kk + kpos) * 128 + p_e
    col = ioff_arr[ci_e, win_e] + j // 16
    row = j % 16

    gidx = np.full((NC, 16, ftot), np.int16(NLR), np.int16)  # pad: zero row
    gidx[core_e, row, col] = rel_s_s.astype(np.int16)

    deg_inv = (1.0 / np.maximum(deg_in, 1)).astype(np.float32)
    deg_inv_perm = np.zeros(V, np.float32)
    deg_inv_perm[gperm] = deg_inv
    deg_inv_perm[orig_of < 0] = 1.0

    return dict(gperm=gperm, orig_of=orig_of, chunks=chunks,
                call_meta=call_meta, ftot=ftot, gidx=gidx,
                deg_inv_perm=deg_inv_perm)


# ---------------------------------------------------------------- program
def build_program(plan, n_cores=NC):
    chunks = plan["chunks"]
    call_meta = plan["call_meta"]
    ftot = plan["ftot"]

    nc = bacc.Bacc("TRN2", target_bir_lowering=False, debug=False,
                   num_devices=n_cores, num_swdge_queues=4)

    # single packed input per core (each device_put/RPC has ~60-80ms fixed
    # cost on the axon tunnel, so everything ships as one f32 blob):
    # [x low bytes u8 | x high bit-plane u8 | gidx i16 | deg_inv i16 |
    #  weight shard f32 | x per-node scale i16], f32-word offsets. x is
    # 9-bit per-node-scaled; deg_inv and the x scale are 15-bit fixed
    # point; the weights+bias ship sharded 1/8 per core and are
    # AllGathered on device.
    XL_W = NLP * D // 4
    XH_W = NLP * D // 32
    GI_W = 8 * ftot                  # 16*ftot int16 = 8*ftot f32 words
    DG_W = 128 * NT // 2             # i16
    AP_W = 128 * NT // 2             # i16
    xl0 = 0
    xh0 = xl0 + XL_W
    gi0 = xh0 + XH_W
    dg0 = gi0 + GI_W
    ws0 = dg0 + DG_W
    ap0 = ws0 + WSH_W
    totw = ap0 + AP_W
    blob = nc.dram_tensor("blob", [totw, 1], F32, kind="ExternalInput")
    out = nc.dram_tensor("out", [1, NLP], F16, kind="ExternalOutput")

    with tile.TileContext(nc) as tc:
        with tc.tile_pool(name="const", bufs=1) as constp, \
             tc.tile_pool(name="hload", bufs=3) as hloadp, \
             tc.tile_pool(name="unp", bufs=1) as unp, \
             tc.tile_pool(name="msg", bufs=4) as msgp, \
             tc.tile_pool(name="part", bufs=4) as partp, \
             tc.tile_pool(name="agg", bufs=3) as aggp, \
             tc.tile_pool(name="rhs", bufs=3) as rhsp, \
             tc.tile_pool(name="zsb", bufs=3) as zsbp, \
             tc.tile_pool(name="zN", bufs=2) as zNp, \
             tc.tile_pool(name="psA", bufs=2, space="PSUM") as psA, \
             tc.tile_pool(name="psB", bufs=2, space="PSUM") as psB, \
             tc.tile_pool(name="psC", bufs=2, space="PSUM") as psC, \
             tc.tile_pool(name="psD", bufs=2, space="PSUM") as psD, \
             tc.tile_pool(name="dram", bufs=1, space="DRAM") as dramp:

            identf = constp.tile([128, 128], F32)
            make_identity(nc, identf[:])

            # weights+bias arrive sharded 1/8 per core; AllGather them
            wsh = dramp.tile([WSH_W, 1], F32, name="wsh")
            wfull = dramp.tile([WB_W, 1], F32, name="wfull",
                               addr_space="Shared")
            nc.sync.dma_start(out=wsh[:], in_=blob.ap()[ws0:ws0 + WSH_W, :])
            nc.gpsimd.collective_compute(
                "AllGather", ALU.bypass,
                replica_groups=[list(range(n_cores))],
                ins=[wsh.opt()], outs=[wfull.opt()],
            )
            wstk_sb = constp.tile([D, 8 * D], F32)
            nc.sync.dma_start(
                out=wstk_sb[:],
                in_=wfull[0:D * 8 * D, :].rearrange(
                    "(r c) o -> r (c o)", r=D))
            bias_sb = constp.tile([D, 4], F32)
            nc.sync.dma_start(
                out=bias_sb[:],
                in_=wfull[D * 8 * D:WB_W, :].rearrange(
                    "(r c) o -> r (c o)", r=D))

            dgc_i16 = constp.tile([128, NT], I16)
            nc.sync.dma_start(
                out=dgc_i16[:],
                in_=blob.ap()[dg0:dg0 + DG_W, :].bitcast(I16).rearrange(
                    "(p th) two -> p (th two)", p=128))
            dgc_sb = constp.tile([128, NT], F32)
            nc.vector.tensor_copy(out=dgc_sb[:], in_=dgc_i16[:])
            nc.vector.tensor_scalar(
                out=dgc_sb[:], in0=dgc_sb[:], scalar1=1.0 / 32767.0,
                scalar2=None, op0=ALU.mult)

            # gather-index table: load 16 rows, replicated to 128
            gidx_ap = blob.ap()[gi0:gi0 + GI_W, :].bitcast(I16).rearrange(
                "(r ch) two -> r (ch two)", r=16)
            idx_sb = constp.tile([128, ftot], I16)
            for k in range(8):
                nc.sync.dma_start(out=idx_sb[16 * k:16 * (k + 1), :],
                                  in_=gidx_ap)

            # deg_inv broadcast [128, NT] -> [128, NT*D]; deg_inv > 0 so
            # Relu(0*x + deginv) == deginv (Copy rejects AP bias)
            dgb = constp.tile([128, NT * D], F32)
            for t in range(NT):
                nc.scalar.activation(
                    out=dgb[:, t * D:(t + 1) * D], in_=identf[:, 0:D],
                    func=AF.Relu, scale=0.0, bias=dgc_sb[:, t:t + 1])

            # x per-node dequant scale (a > 0), 15-bit fixed point
            apk_i16 = constp.tile([128, NT], I16)
            nc.sync.dma_start(
                out=apk_i16[:],
                in_=blob.ap()[ap0:ap0 + AP_W, :].bitcast(I16).rearrange(
                    "(p th) two -> p (th two)", p=128))
            apk_sb = constp.tile([128, NT], F32)
            nc.vector.tensor_copy(out=apk_sb[:], in_=apk_i16[:])
            nc.vector.tensor_scalar(
                out=apk_sb[:], in0=apk_sb[:], scalar1=A16 / 32767.0,
                scalar2=None, op0=ALU.mult)

            agins = [dramp.tile([NLP, D], F32, name=f"agin{i}")
                     for i in range(2)]
            tabs = [dramp.tile([V, D], F32, name=f"tab{i}",
                               addr_space="Shared") for i in range(4)]

            # unpack 12-bit x -> f32 agin1, AllGather -> tab0
            U8 = mybir.dt.uint8
            CH = 14
            QD = D // 8
            g0 = 0
            while g0 < NT:
                Tg = min(CH, NT - g0)
                l8 = unp.tile([128, CH * D], U8, tag="l8")
                nc.sync.dma_start(
                    out=l8[:, :Tg * D].rearrange("p (t f) -> p t f", t=Tg),
                    in_=blob.ap()[xl0 + g0 * 2048:
                                  xl0 + (g0 + Tg) * 2048, :].bitcast(
                        U8).rearrange("(t p fq) four -> p t (fq four)",
                                      p=128, fq=D // 4),
                )
                h8 = unp.tile([128, CH * QD], U8, tag="h8")
                nc.sync.dma_start(
                    out=h8[:, :Tg * QD].rearrange("p (t f) -> p t f", t=Tg),
                    in_=blob.ap()[xh0 + g0 * 256:
                                  xh0 + (g0 + Tg) * 256, :].bitcast(
                        U8).rearrange("(t p fq) four -> p t (fq four)",
                                      p=128, fq=D // 32),
                )
                l16 = unp.tile([128, CH * D], I16, tag="l16")
                nc.vector.tensor_copy(out=l16[:, :Tg * D],
                                      in_=l8[:, :Tg * D])
                h16 = unp.tile([128, CH * QD], I16, tag="h16")
                nc.vector.tensor_copy(out=h16[:, :Tg * QD],
                                      in_=h8[:, :Tg * QD])
                qs = []
                for k in range(8):
                    qk = unp.tile([128, CH * QD], I16, tag=f"q{k}")
                    nc.vector.tensor_scalar(
                        out=qk[:, :Tg * QD], in0=h16[:, :Tg * QD],
                        scalar1=k, scalar2=None,
                        op0=ALU.logical_shift_right)
                    nc.vector.tensor_scalar(
                        out=qk[:, :Tg * QD], in0=qk[:, :Tg * QD],
                        scalar1=1, scalar2=None, op0=ALU.bitwise_and)
                    nc.vector.tensor_scalar(
                        out=qk[:, :Tg * QD], in0=qk[:, :Tg * QD],
                        scalar1=8, scalar2=None,
                        op0=ALU.logical_shift_left)
                    qs.append(qk)
                v16 = unp.tile([128, CH * D], I16, tag="v16")
                for t in range(Tg):
                    for k in range(8):
                        nc.vector.tensor_tensor(
                            out=v16[:, t * D + k * QD:t * D + (k + 1) * QD],
                            in0=l16[:, t * D + k * QD:t * D + (k + 1) * QD],
                            in1=qs[k][:, t * QD:(t + 1) * QD], op=ALU.add)
                xc = hloadp.tile([128, CH * D], F32, tag="hload")
                nc.vector.tensor_copy(out=xc[:, :Tg * D],
                                      in_=v16[:, :Tg * D])
                nc.vector.tensor_scalar(
                    out=xc[:, :Tg * D], in0=xc[:, :Tg * D],
                    scalar1=-256.0, scalar2=None, op0=ALU.add)
                abr_c = unp.tile([128, CH * D], F32, tag="abr_c")
                for t in range(Tg):
                    nc.scalar.activation(
                        out=abr_c[:, t * D:(t + 1) * D], in_=identf[:, 0:D],
                        func=AF.Relu, scale=0.0,
                        bias=apk_sb[:, g0 + t:g0 + t + 1])
                nc.vector.tensor_tensor(
                    out=xc[:, :Tg * D], in0=xc[:, :Tg * D],
                    in1=abr_c[:, :Tg * D], op=ALU.mult)
                nc.sync.dma_start(
                    out=agins[1][g0 * 128:(g0 + Tg) * 128, :].rearrange(
                        "(t p) f -> p t f", p=128),
                    in_=xc[:, :Tg * D].rearrange("p (t f) -> p t f", t=Tg),
                )
                g0 += Tg
            nc.gpsimd.collective_compute(
                "AllGather", ALU.bypass,
                replica_groups=[list(range(n_cores))],
                ins=[agins[1].opt()], outs=[tabs[0].opt()],
            )

            ci_meta = {}
            for m in call_meta:
                ci_meta.setdefault(m["chunk"], []).append(m)

            def layer(l, tab, agin_prev, agin_out):
                last = l == 3
                MOUT = 1 if last else D
                wself = wstk_sb[:, 2 * l * D:2 * l * D + MOUT]
                wneigh = wstk_sb[:, (2 * l + 1) * D:(2 * l + 1) * D + MOUT]
                bias_ap = bias_sb[0:MOUT, l:l + 1]

                for ci, (tile0, T, K) in enumerate(chunks):
                    ms = ci_meta[ci]
                    # self rows (f32, node-major) from previous layer
                    hload = hloadp.tile([128, T * D], F32, tag="hload")
                    nc.sync.dma_start(
                        out=hload[:].rearrange("p (t f) -> p t f", t=T),
                        in_=agin_prev[tile0 * 128:(tile0 + T) * 128,
                                      :].rearrange("(t p) f -> p t f",
                                                   p=128),
                    )

                    part = partp.tile([128, W * T * D], F32, tag="part")
                    for m in ms:
                        w, Kw, n_idx = m["w"], m["K"], m["n_idx"]
                        if Kw == 0:
                            nc.vector.memset(
                                part[:, w * T * D:(w + 1) * T * D], 0.0)
                            continue
                        # per-window msg tiles: window w of chunk n+2 only
                        # waits on its own reduce from chunk n
                        msg = msgp.tile([128, T * Kw * D], F32,
                                        tag=f"msg{w}")
                        nc.gpsimd.dma_gather(
                            msg[:].rearrange(
                                "p (s e) -> p s e", s=T * Kw, e=D),
                            tab[w * WINP:(w + 1) * WINP, :],
                            idx_sb[:, m["ioff"]:m["ioff"] + n_idx // 16],
                            n_idx, n_idx, D, elem_step=D,
                            queue_num=w, single_packet=False,
                        )
                        nc.vector.tensor_reduce(
                            out=part[:, w * T * D:(w + 1) * T * D].rearrange(
                                "p (t e) -> p t e", t=T, e=D),
                            in_=msg[:].rearrange(
                                "p (t k e) -> p t e k", t=T, k=Kw, e=D),
                            axis=mybir.AxisListType.X, op=ALU.add,
                        )

                    agg = aggp.tile([128, T * D], F32, tag="agg")
                    nc.vector.tensor_reduce(
                        out=agg[:], in_=part[:].rearrange(
                            "p (w s) -> p s w", w=W, s=T * D),
                        axis=mybir.AxisListType.X, op=ALU.add,
                    )
                    nc.vector.tensor_tensor(
                        out=agg[:], in0=agg[:],
                        in1=dgb[:, tile0 * D:(tile0 + T) * D],
                        op=ALU.mult,
                    )

                    bt = 0
                    while bt < T:
                        Tb = min(4, T - bt)
                        cols0 = (tile0 + bt) * 128
                        cols = slice(cols0, cols0 + Tb * 128)
                        hT_ps = psA.tile([D, Tb * 128], F32, tag="hT_ps")
                        aT_ps = psC.tile([D, Tb * 128], F32, tag="aT_ps")
                        for tt in range(Tb):
                            nc.tensor.transpose(
                                out=hT_ps[:, tt * 128:(tt + 1) * 128],
                                in_=hload[:, (bt + tt) * D:
                                          (bt + tt + 1) * D],
                                identity=identf[:],
                            )
                            nc.tensor.transpose(
                                out=aT_ps[:, tt * 128:(tt + 1) * 128],
                                in_=agg[:, (bt + tt) * D:(bt + tt + 1) * D],
                                identity=identf[:],
                            )
                        hT_sb = rhsp.tile([D, Tb * 128], F32, tag="hT_sb")
                        aT_sb = rhsp.tile([D, Tb * 128], F32, tag="aT_sb")
                        nc.vector.tensor_copy(out=hT_sb[:], in_=hT_ps[:])
                        nc.vector.tensor_copy(out=aT_sb[:], in_=aT_ps[:])

                        z_ps = psB.tile([MOUT, Tb * 128], F32, tag="z_ps")
                        nc.tensor.matmul(out=z_ps[:], lhsT=wself,
                                         rhs=hT_sb[:], start=True,
                                         stop=False)
                        nc.tensor.matmul(out=z_ps[:], lhsT=wneigh,
                                         rhs=aT_sb[:], start=False,
                                         stop=True)
                        if last:
                            osb = zsbp.tile([1, 512], F16, tag="osb")
                            nc.scalar.activation(
                                out=osb[0:1, :Tb * 128], in_=z_ps[:],
                                func=AF.Sigmoid, bias=bias_ap)
                            nc.sync.dma_start(out=out.ap()[0:1, cols],
                                              in_=osb[0:1, :Tb * 128])
                        else:
                            zsb = zsbp.tile([64, Tb * 128], F32, tag="zsb")
                            nc.scalar.activation(
                                out=zsb[:], in_=z_ps[:],
                                func=AF.Relu, bias=bias_ap)
                            if cols0 + Tb * 128 > NLR:
                                pad0 = max(0, NLR - cols0)
                                nc.vector.memset(zsb[:, pad0:Tb * 128], 0.0)
                            zT_ps = psD.tile([128, Tb * D], F32, tag="zT")
                            for tt in range(Tb):
                                nc.tensor.transpose(
                                    out=zT_ps[:, tt * D:(tt + 1) * D],
                                    in_=zsb[:, tt * 128:(tt + 1) * 128],
                                    identity=identf[:64, :64],
                                )
                            zN = zNp.tile([128, Tb * D], F32, tag="zN")
                            nc.vector.tensor_copy(out=zN[:], in_=zT_ps[:])
                            nc.sync.dma_start(
                                out=agin_out[cols0:cols0 + Tb * 128,
                                             :].rearrange(
                                    "(t p) f -> p t f", t=Tb, p=128),
                                in_=zN[:].rearrange("p (t f) -> p t f",
                                                    t=Tb, f=D),
                            )
                        bt += Tb

                if not last:
                    nc.gpsimd.collective_compute(
                        "AllGather", ALU.bypass,
                        replica_groups=[list(range(n_cores))],
                        ins=[agin_out.opt()], outs=[tabs[l + 1].opt()],
                    )

            layer(0, tabs[0][:], agins[1], agins[0])
            layer(1, tabs[1][:], agins[0], agins[1])
            layer(2, tabs[2][:], agins[1], agins[0])
            layer(3, tabs[3][:], agins[0], None)

    nc.compile()
    return nc


# ------------------------------------------------------------------ driver
def make_in_maps(plan, x, Wself1, Wneigh1, b1, Wself2, Wneigh2, b2,
                 Wself3, Wneigh3, b3, Wself4, Wneigh4, b4):
    gperm = plan["gperm"]
    xtab = np.zeros((V, D), np.float32)
    xtab[gperm] = np.asarray(x, np.float32)
    deginv = plan["deg_inv_perm"]

    wstk = np.zeros((D, 8 * D), np.float32)
    for l, (ws, wn) in enumerate(((Wself1, Wneigh1), (Wself2, Wneigh2),
                                  (Wself3, Wneigh3), (Wself4, Wneigh4))):
        ws = np.asarray(ws, np.float32)
        wn = np.asarray(wn, np.float32)
        wstk[:, 2 * l * D:2 * l * D + ws.shape[1]] = ws
        wstk[:, (2 * l + 1) * D:(2 * l + 1) * D + wn.shape[1]] = wn
    bias = np.zeros((D, 4), np.float32)
    for l, b in enumerate((b1, b2, b3, b4)):
        b = np.asarray(b, np.float32)
        bias[:, l] = b[0] if b.shape[0] == 1 else b

    ftot = plan["ftot"]
    XL_W = NLP * D // 4
    XH_W = NLP * D // 32
    GI_W = 8 * ftot
    DG_W = 128 * NT // 2
    AP_W = 128 * NT // 2
    totw = XL_W + XH_W + GI_W + DG_W + WSH_W + AP_W

    wb = np.concatenate([wstk.reshape(-1), bias.reshape(-1)])
    assert wb.size == WB_W

    in_maps = []
    for c in range(NC):
        sl = slice(c * NLP, (c + 1) * NLP)
        xsf = xtab[sl]
        # 9-bit per-node-scaled x; scale is 15-bit fixed point of A16
        amax = np.maximum(np.abs(xsf).max(axis=1), 1e-12)
        au = np.clip(np.round(amax / 16.0 * 32767.0), 1, 32767)
        a = (au.astype(np.float32) * np.float32(A16 / 32767.0))
        vq = (np.clip(np.round(xsf / a[:, None]), -255, 255)
              + 256).astype(np.uint16)
        xl = (vq & 0xFF).astype(np.uint8)
        hi = (vq >> 8).astype(np.uint8)          # 1 bit
        Q = D // 8
        xh = np.zeros((xsf.shape[0], Q), np.uint8)
        for k in range(8):
            xh |= (hi[:, k * Q:(k + 1) * Q] << k).astype(np.uint8)
        apk = np.ascontiguousarray(
            au.reshape(NT, 128).T).astype(np.int16)
        dgc = np.ascontiguousarray(
            np.clip(np.round(deginv[sl] * 32767.0), 1, 32767
                    ).reshape(NT, 128).T).astype(np.int16)
        wsh = wb[c * WSH_W:(c + 1) * WSH_W]
        blob = np.empty(totw * 4, np.uint8)
        o = 0
        for arr in (xl, xh, plan["gidx"][c], dgc, wsh, apk):
            b = np.ascontiguousarray(arr).view(np.uint8).reshape(-1)
            blob[o:o + b.size] = b
            o += b.size
        assert o == totw * 4
        in_maps.append(dict(blob=blob.view(np.float32).reshape(totw, 1)))
    return in_maps


def kernel(x, edge_index, Wself1, Wneigh1, b1, Wself2, Wneigh2, b2,
           Wself3, Wneigh3, b3, Wself4, Wneigh4, b4):
    edge_index = np.asarray(edge_index)
    plan = build_plan(edge_index)
    in_maps = make_in_maps(plan, x, Wself1, Wneigh1, b1, Wself2, Wneigh2,
                           b2, Wself3, Wneigh3, b3, Wself4, Wneigh4, b4)
    nc = build_program(plan, n_cores=NC)
    res = run_bass_kernel_spmd(nc, in_maps, core_ids=list(range(NC)))

    out_perm = np.concatenate(
        [np.asarray(res.results[c]["out"]).reshape(-1)[:NLR].astype(
            np.float32) for c in range(NC)])
    orig = np.concatenate([plan["orig_of"][c * NLP:c * NLP + NLR]
                           for c in range(NC)])
    out_full = np.empty(N, np.float32)
    out_full[orig] = out_perm
    return out_full.reshape(N, 1)

